# revision 1
# baseline (speedup 1.0000x reference)
"""MoE-routed dynamic conv kernel for Trainium2 (8 NeuronCores, SPMD).

Problem: per-sample attention (global avg pool -> 1x1 conv -> sigmoid) mixes
K=4 expert 3x3 conv kernels; each sample is convolved with its own mixed
kernel.  x: (32, 256, 56, 56), att_w: (4, 256), weight: (4, 256, 256, 3, 3).

Strategy: data parallel over batch (4 samples per core, weights replicated).
x is zero-padded to (58, 58) on the host, so on device every conv tap
(kh, kw) is a flat contiguous slice of the padded image.  Per sample:
  - pooled sums via DVE free-dim reduce over the padded x tile
  - attention logits via tiny f32 PE matmuls against a host-side replicated
    att_w (gives att_k broadcast across all 128 partitions), sigmoid on ACT
  - expert mixing (agg = sum_k att_k * w_k) via 4 fused DVE ops per ci-block
  - conv as implicit GEMM in fp32r (FP22-truncated reads, full PE rate,
    even-count/aligned APs per the fp32r ISA restrictions): 18 matmuls
    (9 taps x 2 ci-blocks) accumulate into each PSUM chunk of 464 output
    columns (8 rows x 58); the two padded columns per row are discarded by
    the strided output DMA.

The per-sample stages are software-pipelined (att/mix of sample b+1 is
emitted before the conv of sample b) so the PE never waits on the
attention -> sigmoid -> mixing chain at sample boundaries.
"""

import sys

if "/opt/trn_rl_repo" not in sys.path:
    sys.path.insert(0, "/opt/trn_rl_repo")

import numpy as np

B_TOTAL = 32
N_CORES = 8
B_PER_CORE = B_TOTAL // N_CORES  # 4
CI = 256
CO = 256
K = 4
H = W = 56
PH = PW = 58
FLAT = PH * PW            # 3364 padded image
XT_F = FLAT + 4           # 3368: + tail pad for tap (2,2) overrun, host zeros
OUTF = H * W              # 3136 output cols per co-block (contiguous)
RPC = 8                   # output rows per PSUM chunk
NCHUNK = RPC * W          # 448 = 8 rows x 56 valid cols (even, aligned)
NCHUNKS = H // RPC        # 7
TAPS = 9
TPC = TAPS * CO           # 2304 free elems per (k, ci-block) weight tile

_cache = {}


def _build_nc():
    from contextlib import ExitStack

    import concourse.bacc as bacc
    import concourse.mybir as mybir
    import concourse.tile as tile

    f32 = mybir.dt.float32
    f32r = mybir.dt.float32r
    AF = mybir.ActivationFunctionType
    ALU = mybir.AluOpType

    nc = bacc.Bacc("TRN2", target_bir_lowering=False, debug=False)
    x_p = nc.declare_dram_parameter("x", [B_PER_CORE, CI, XT_F], f32r, isOutput=False)
    w_p = nc.declare_dram_parameter("w", [K, CI, 3, 3, CO], f32, isOutput=False)
    ar_p = nc.declare_dram_parameter("attrep", [CI, K * 128], f32, isOutput=False)
    o_p = nc.declare_dram_parameter("out", [B_PER_CORE, CO, H, W], f32, isOutput=True)

    with ExitStack() as ctx:
        tc = ctx.enter_context(tile.TileContext(nc))
        pw = ctx.enter_context(tc.tile_pool(name="wpool", bufs=1))
        px = ctx.enter_context(tc.tile_pool(name="xpool", bufs=4))
        pagg = ctx.enter_context(tc.tile_pool(name="aggpool", bufs=4))
        pout = ctx.enter_context(tc.tile_pool(name="outpool", bufs=2))
        psml = ctx.enter_context(tc.tile_pool(name="small", bufs=4))
        pps = ctx.enter_context(tc.tile_pool(name="cpsum", bufs=7, space="PSUM"))
        ppsa = ctx.enter_context(tc.tile_pool(name="apsum", bufs=1, space="PSUM"))

        # Replicated attention weights (col j of block k = att_w[k, :]) and
        # the resident expert weights, free layout (k, tap, co) per ci-block.
        ar_sb = []
        for c in range(2):
            at = pw.tile([128, K * 128], f32, tag=f"ar{c}")
            nc.sync.dma_start(out=at[:, :], in_=ar_p[c * 128 : (c + 1) * 128, :])
            ar_sb.append(at)
        w_sb = [
            pw.tile([128, K * TPC], f32, tag=f"w{c}", name=f"wt{c}")
            for c in range(2)
        ]
        for k in range(K):
            for c in range(2):
                nc.sync.dma_start(
                    out=w_sb[c][:, k * TPC : (k + 1) * TPC],
                    in_=w_p[k, c * 128 : (c + 1) * 128].rearrange(
                        "ci kh kw co -> ci (kh kw co)"
                    ),
                )

        state = {}

        def stage_load(b):
            """Load padded x_b (both HWDGE engines) and pool."""
            xts = []
            pooleds = []
            for c in range(2):
                xt = px.tile([128, XT_F], f32r, tag="x")
                eng = nc.scalar if c == 0 else nc.gpsimd
                eng.dma_start(out=xt[:, :], in_=x_p[b, c * 128 : (c + 1) * 128, :])
                pl = psml.tile([128, 1], f32, tag="pooled")
                nc.vector.tensor_reduce(
                    pl[:, :], xt[:, 0:FLAT], axis=mybir.AxisListType.X, op=ALU.add
                )
                xts.append(xt)
                pooleds.append(pl)
            state[b] = (xts, pooleds)

        def stage_att(b):
            """Attention matmuls + sigmoid for sample b."""
            xts, pooleds = state[b]
            # att[k] broadcast over all partitions: lhsT column j = att_w[k,:]
            # for every j, so out[j, 0] = dot(att_w[k], pooled) for all j.
            # Plain f32 matmuls (fp32r forbids odd moving counts like N=1).
            att_ps = ppsa.tile([128, K], f32, tag="attps")
            for k in range(K):
                for c in range(2):
                    nc.tensor.matmul(
                        att_ps[:, k : k + 1],
                        lhsT=ar_sb[c][:, k * 128 : (k + 1) * 128],
                        rhs=pooleds[c][:, :],
                        start=(c == 0),
                        stop=(c == 1),
                    )
            att_sb = psml.tile([128, K], f32, tag="attsb")
            nc.scalar.activation(
                att_sb[:, :], att_ps[:, :], AF.Sigmoid, scale=1.0 / (H * W)
            )
            state[b] = (xts, att_sb)

        def stage_mix(b):
            """Expert mixing on DVE: agg = sum_k att_k * w_k, fused mul-add."""
            xts, att_sb = state[b]
            aggs = []
            for c in range(2):
                ag = pagg.tile([128, TPC], f32r, tag="agg")
                nc.vector.tensor_scalar_mul(ag[:, :], w_sb[c][:, 0:TPC], att_sb[:, 0:1])
                for k in range(1, K):
                    nc.vector.scalar_tensor_tensor(
                        ag[:, :],
                        w_sb[c][:, k * TPC : (k + 1) * TPC],
                        att_sb[:, k : k + 1],
                        ag[:, :],
                        ALU.mult,
                        ALU.add,
                    )
                aggs.append(ag)
            state[b] = (xts, aggs)


        def stage_b(b, after_first_chunk=None):
            """Conv for sample b: per co-block, 7 PSUM chunks of 8x56 cols.

            Each tap is a 2D window [8 rows, 56 valid cols] of the padded
            image (row stride 58), written to a dense [8, 56] PSUM chunk:
            no wasted pad columns, and the output buffer stays contiguous.
            """
            xts, aggs = state.pop(b)
            x3s = [
                xt[:, :FLAT].rearrange("p (h w) -> p h w", h=PH) for xt in xts
            ]
            first_chunk_done = False
            for cb in range(2):
                osb = pout.tile([128, OUTF], f32, tag="osb")
                for ch in range(NCHUNKS):
                    r0 = ch * RPC
                    ps = pps.tile([128, NCHUNK], f32, tag="convps")
                    ps3 = ps[:, :].rearrange("p (h w) -> p h w", h=RPC)
                    i = 0
                    for c in range(2):
                        for t in range(TAPS):
                            dr, dc = t // 3, t % 3
                            nc.tensor.matmul(
                                ps3[:, :, :],
                                lhsT=aggs[c][
                                    :, t * CO + cb * 128 : t * CO + cb * 128 + 128
                                ],
                                rhs=x3s[c][
                                    :, r0 + dr : r0 + dr + RPC, dc : dc + W
                                ],
                                start=(i == 0),
                                stop=(i == 17),
                            )
                            i += 1
                    nc.scalar.copy(osb[:, ch * NCHUNK : (ch + 1) * NCHUNK], ps[:, :])
                    if not first_chunk_done:
                        first_chunk_done = True
                        if after_first_chunk is not None:
                            after_first_chunk()
                # Contiguous output; split in halves so the first transfer
                # overlaps the remaining evictions.
                half = OUTF // 2
                for s in range(2):
                    nc.sync.dma_start(
                        out=o_p[b, cb * 128 : (cb + 1) * 128].rearrange(
                            "co h w -> co (h w)"
                        )[:, s * half : (s + 1) * half],
                        in_=osb[:, s * half : (s + 1) * half],
                    )

        # Software pipeline: loads lead their mix; att/mix of b+1 precedes
        # conv of b so the PE never stalls on the attention chain.
        stage_load(0)
        stage_att(0)
        stage_mix(0)
        stage_load(1)

        def _att_mix_1():
            stage_att(1)
            stage_mix(1)

        stage_b(0, after_first_chunk=_att_mix_1)
        stage_load(2)
        stage_att(2)
        stage_mix(2)
        stage_b(1)
        stage_load(3)
        stage_att(3)
        stage_mix(3)
        stage_b(2)
        stage_b(3)

    nc.compile()
    return nc


def _get_nc():
    if "nc" not in _cache:
        _cache["nc"] = _build_nc()
    return _cache["nc"]


def _make_in_maps(x, att_w, weight):
    x = np.asarray(x, dtype=np.float32)
    att_w = np.asarray(att_w, dtype=np.float32)
    weight = np.asarray(weight, dtype=np.float32)
    # Host-side zero pad to (58, 58) + 4 tail elems, flattened per channel.
    xp = np.zeros((B_TOTAL, CI, XT_F), dtype=np.float32)
    xp[:, :, :FLAT] = np.pad(
        x, ((0, 0), (0, 0), (1, 1), (1, 1))
    ).reshape(B_TOTAL, CI, FLAT)
    # (K, Cout, Cin, kh, kw) -> (K, Cin, kh, kw, Cout) so the SBUF lhsT
    # layout [ci, (tap, co)] is a contiguous DMA.
    w_t = np.ascontiguousarray(weight.transpose(0, 2, 3, 4, 1))
    # (Cin, K*128): col j = att_w[j // 128, ci]
    att_rep = np.ascontiguousarray(np.repeat(att_w.T, 128, axis=1))
    return [
        {
            "x": np.ascontiguousarray(xp[i * B_PER_CORE : (i + 1) * B_PER_CORE]),
            "w": w_t,
            "attrep": att_rep,
        }
        for i in range(N_CORES)
    ]


def _run(x, att_w, weight, trace=False, **spmd_kwargs):
    from concourse.bass_utils import run_bass_kernel_spmd

    nc = _get_nc()
    in_maps = _make_in_maps(x, att_w, weight)
    res = run_bass_kernel_spmd(
        nc, in_maps, list(range(N_CORES)), trace=trace, **spmd_kwargs
    )
    out = np.concatenate([r["out"] for r in res.results], axis=0)
    return out.astype(np.float32, copy=False), res


def kernel(x, att_w, weight):
    out, _ = _run(x, att_w, weight)
    return out



# revision 12
# speedup vs baseline: 1.3534x; 1.3534x over previous
"""MoE-routed dynamic conv kernel for Trainium2 (8 NeuronCores, SPMD).

Problem: per-sample attention (global avg pool -> 1x1 conv -> sigmoid) mixes
K=4 expert 3x3 conv kernels; each sample is convolved with its own mixed
kernel.  x: (32, 256, 56, 56), att_w: (4, 256), weight: (4, 256, 256, 3, 3).

Strategy: data parallel over batch (4 samples per core, weights replicated).
The conv runs as an implicit GEMM in double-pumped fp8 (DoubleRow perf mode,
0.5 PE cycles per output row): each matmul instruction carries TWO
(weights, moving) fp8 pairs that are multiplied and summed in one pass, so
the two ci-blocks (ci 0-127 / 128-255) are contracted together.

Plain e4m3 quantization of both operands is too lossy (~3.5e-2 rel err vs
the 2e-2 gate), so both sides carry an exact hi+lo e4m3 split at a shared
power-of-2 scale (x*32, w*512) and each tap issues three DoubleRow matmuls:
    (w8 . x8), (w8 . xlo), (wlo . x8)        [wlo.xlo dropped, O(1e-3) rel]
i.e. 13.5 fp32r-equivalent passes instead of 18 -> ~0.75x PE time, with
measured end-to-end rel err ~1e-3.

x is zero-padded to (58, 58) on the host, quantized there into x8/xlo e4m3
planes laid out [p, two, flat] (two = ci-block), so every conv tap is a
contiguous [128, 2, 464] slice (8 padded rows); the two pad columns per row
are dropped by the strided PSUM->SBUF eviction, keeping the output DMA
contiguous.

Scheduling notes (the Tile scheduler list-schedules each engine greedily by
ready time; emission order only fixes ties, so the structure below shapes
READY TIMES, not queue positions):
  - The mix (agg = sum_k att_k w_k, bf16 mul/add tree on DVE; the fused
    scalar_tensor_tensor form runs 1x, tensor_scalar_mul 4x / tensor_tensor
    2x) and the agg8/agglo8 e4m3 hi/lo split are pipelined in TAP-THIRDS,
    and the hi conv passes run tap-outer, so sample 0's conv starts after
    only a third of the mix chain.
  - Pooling for sample 0 splits ci-block 0 to ACT (activation accum_out) /
    block 1 to a DVE reduce that finishes before the mix begins; for b>=1
    both halves run on ACT and are GATED behind agg8(b-1) by a tiny
    dependency op, or the greedy scheduler slots their 3.2us ops into the
    critical sigmoid->mix->agg8 chain of the previous sample.
  - No DMAs ride the ACT queue (its DGE path costs ~50% extra and blocks
    the in-order pool/sigmoid/evict stream).
  - A dummy sigmoid at t=0 preloads the activation-function table so the
    1.3us LoadActFuncSet is off the first sample's chain.
"""

import sys

if "/opt/trn_rl_repo" not in sys.path:
    sys.path.insert(0, "/opt/trn_rl_repo")

import numpy as np

B_TOTAL = 32
N_CORES = 8
B_PER_CORE = B_TOTAL // N_CORES  # 4
CI = 256
CO = 256
K = 4
H = W = 56
PH = PW = 58
FLAT = PH * PW            # 3364 padded image per ci-block
XT_F = FLAT + 4           # 3368: + tail pad for tap (2,2) overrun, host zeros
TWO = 2                   # ci-blocks paired per DoubleRow matmul
OUTF = H * W              # 3136 output cols per co-block (contiguous)
RPC = 8                   # output rows per PSUM chunk
CW = RPC * PW             # 464 = 8 padded rows; 2 pad cols/row discarded
NCHUNKS = H // RPC        # 7
TAPS = 9
NT = 3                    # mix/quant pipeline stages (tap-thirds)
TPT = TAPS // NT          # 3 taps per third
THF = TPT * TWO * CO      # 1536 free elems per third
MIXF = TAPS * TWO * CO    # 4608 free elems per mixed-weight tile
SX = 32.0                 # x quant scale (power of 2; |x*32| < 240)
SW = 512.0                # w quant scale (power of 2; |agg*512| < 240)

_cache = {}


def _build_nc():
    from contextlib import ExitStack

    import concourse.bacc as bacc
    import concourse.mybir as mybir
    import concourse.tile as tile

    f32 = mybir.dt.float32
    bf16 = mybir.dt.bfloat16
    fp8 = mybir.dt.float8e4
    AF = mybir.ActivationFunctionType
    ALU = mybir.AluOpType
    DR = mybir.MatmulPerfMode.DoubleRow

    nc = bacc.Bacc("TRN2", target_bir_lowering=False, debug=False)
    x8_p = nc.declare_dram_parameter(
        "x8", [B_PER_CORE, 128, TWO * XT_F], fp8, isOutput=False
    )
    xlo_p = nc.declare_dram_parameter(
        "xlo", [B_PER_CORE, 128, TWO * XT_F], fp8, isOutput=False
    )
    w_p = nc.declare_dram_parameter("w", [K, 128, MIXF], bf16, isOutput=False)
    ar_p = nc.declare_dram_parameter("attrep", [128, TWO * K * 128], f32, isOutput=False)
    o_p = nc.declare_dram_parameter("out", [B_PER_CORE, CO, H, W], f32, isOutput=True)

    with ExitStack() as ctx:
        tc = ctx.enter_context(tile.TileContext(nc))
        pw = ctx.enter_context(tc.tile_pool(name="wpool", bufs=1))
        px = ctx.enter_context(tc.tile_pool(name="xpool", bufs=3))
        pagg = ctx.enter_context(tc.tile_pool(name="aggpool", bufs=2))
        pq = ctx.enter_context(tc.tile_pool(name="qpool", bufs=2))
        pout = ctx.enter_context(tc.tile_pool(name="outpool", bufs=2))
        psml = ctx.enter_context(tc.tile_pool(name="small", bufs=4))
        pps = ctx.enter_context(tc.tile_pool(name="cpsum", bufs=7, space="PSUM"))
        ppsa = ctx.enter_context(tc.tile_pool(name="apsum", bufs=1, space="PSUM"))

        ar_sb = pw.tile([128, TWO * K * 128], f32, tag="ar")
        w_sb = pw.tile([128, K * MIXF], bf16, tag="w", name="wt")
        # Mix scratch (bf16): reused across samples, WAR-serialized on DVE.
        m0 = pw.tile([128, MIXF], bf16, tag="m0")
        m1 = pw.tile([128, MIXF], bf16, tag="m1")
        psc = pw.tile([128, FLAT], fp8, tag="poolscr")
        warm = pw.tile([128, 2], f32, tag="warm")

        def warm_act():
            nc.vector.memset(warm[:, :], 0.0)
            nc.scalar.activation(warm[:, :], warm[:, :], AF.Sigmoid)

        def load_weights():
            nc.sync.dma_start(out=ar_sb[:, :], in_=ar_p[:, :])
            # Third j of expert k lands just before mix op (j, k) needs it.
            engs = (nc.sync, nc.gpsimd, nc.gpsimd, nc.sync)
            for j in range(NT):
                for k in range(K):
                    engs[k].dma_start(
                        out=w_sb[:, k * MIXF + j * THF : k * MIXF + (j + 1) * THF],
                        in_=w_p[k, :, j * THF : (j + 1) * THF],
                    )

        ar3 = ar_sb[:, :].rearrange("p (c rest) -> p c rest", c=TWO)

        state = {}
        agg8_of = {}

        def load_dma(b, x8_eng, xlo_eng, split_x8=False):
            x8t = px.tile([128, TWO * XT_F], fp8, tag="x8")
            xlot = px.tile([128, TWO * XT_F], fp8, tag="xlo")
            if split_x8:
                for c in range(TWO):
                    x8_eng.dma_start(
                        out=x8t[:, c * XT_F : (c + 1) * XT_F],
                        in_=x8_p[b, :, c * XT_F : (c + 1) * XT_F],
                    )
            else:
                x8_eng.dma_start(out=x8t[:, :], in_=x8_p[b, :, :])
            xlo_eng.dma_start(out=xlot[:, :], in_=xlo_p[b, :, :])
            state[b] = [x8t, xlot]

        def load_pool(b):
            """Pool x8 per ci-block.  b=0: ACT + DVE halves in parallel (the
            DVE finishes before the mix begins, so it cannot poison the mix
            chain).  b>=1: both halves on ACT, gated behind agg8(b-1)."""
            x8t, xlot = state[b]
            pl = psml.tile([128, TWO], f32, tag="pooled")
            x3 = x8t[:, :].rearrange("p (two f) -> p two f", two=TWO)
            if b > 0:
                nc.scalar.activation(
                    pl[:, :], agg8_of[b - 1][:, 3071:3073], AF.Copy, scale=0.0
                )
            nc.scalar.activation(
                psc[:, :], x3[:, 0, 0:FLAT], AF.Copy, accum_out=pl[:, 0:1]
            )
            if b == 0:
                nc.vector.tensor_reduce(
                    pl[:, 1:2],
                    x3[:, 1:2, 0:FLAT],
                    axis=mybir.AxisListType.X,
                    op=ALU.add,
                )
            else:
                nc.scalar.activation(
                    psc[:, :], x3[:, 1, 0:FLAT], AF.Copy, accum_out=pl[:, 1:2]
                )
            state[b] = [x8t, xlot, pl]

        def stage_att(b):
            """Attention matmuls + sigmoid for sample b (f32, tiny)."""
            x8t, xlot, pl = state[b]
            att_ps = ppsa.tile([128, K], f32, tag="attps")
            for k in range(K):
                for c in range(TWO):
                    nc.tensor.matmul(
                        att_ps[:, k : k + 1],
                        lhsT=ar3[:, c, k * 128 : (k + 1) * 128],
                        rhs=pl[:, c : c + 1],
                        start=(c == 0),
                        stop=(c == 1),
                    )
            att_sb = psml.tile([128, K], f32, tag="attsb")
            # pooled is computed from x*SX, so fold 1/SX into the logit scale.
            nc.scalar.activation(
                att_sb[:, :], att_ps[:, :], AF.Sigmoid, scale=1.0 / (H * W * SX)
            )
            state[b] = [x8t, xlot, att_sb]

        def stage_mix_quant(b):
            """Per tap-third: bf16 mul/add tree on DVE, then agg8 = e4m3(
            agg*SW) on ACT and agglo8 = agg*SW - agg8 on DVE."""
            x8t, xlot, att_sb = state[b]
            agg = pagg.tile([128, MIXF], bf16, tag="agg")
            agg8 = pq.tile([128, MIXF], fp8, tag="agg8")
            agglo = pq.tile([128, MIXF], fp8, tag="agglo")
            for j in range(NT):
                s = slice(j * THF, (j + 1) * THF)
                wv = [w_sb[:, k * MIXF + j * THF : k * MIXF + (j + 1) * THF]
                      for k in range(K)]
                nc.vector.tensor_scalar_mul(m0[:, s], wv[0], att_sb[:, 0:1])
                nc.vector.tensor_scalar_mul(m1[:, s], wv[1], att_sb[:, 1:2])
                nc.vector.tensor_tensor(agg[:, s], m0[:, s], m1[:, s], ALU.add)
                nc.vector.tensor_scalar_mul(m0[:, s], wv[2], att_sb[:, 2:3])
                nc.vector.tensor_scalar_mul(m1[:, s], wv[3], att_sb[:, 3:4])
                nc.vector.tensor_tensor(m0[:, s], m0[:, s], m1[:, s], ALU.add)
                nc.vector.tensor_tensor(agg[:, s], agg[:, s], m0[:, s], ALU.add)
                nc.scalar.activation(agg8[:, s], agg[:, s], AF.Copy, scale=SW)
                nc.vector.scalar_tensor_tensor(
                    agglo[:, s], agg[:, s], SW, agg8[:, s], ALU.mult, ALU.subtract
                )
            agg8_of[b] = agg8
            state[b] = [x8t, xlot, agg8, agglo]

        def stage_prep(b, xb=None):
            load_pool(b)
            stage_att(b)
            stage_mix_quant(b)
            if xb is not None:
                load_dma(xb, nc.sync, nc.gpsimd)

        def stage_b(b, hook=None):
            """Conv for sample b.

            Per co-block: hi passes (w8 . x8), (w8 . xlo) tap-outer (so the
            first tap-third of agg8 unblocks the PE), then the (wlo . x8)
            passes chunk-major + evictions."""
            x8t, xlot, agg8, agglo = state.pop(b)
            x8v = x8t[:, :].rearrange("p (two f) -> p two f", two=TWO)
            xlov = xlot[:, :].rearrange("p (two f) -> p two f", two=TWO)
            a8v = agg8[:, :].rearrange("p (t two co) -> p t two co", t=TAPS, two=TWO)
            alov = agglo[:, :].rearrange("p (t two co) -> p t two co", t=TAPS, two=TWO)
            for cb in range(2):
                osb = pout.tile([128, OUTF], f32, tag="osb")
                pss = [
                    pps.tile([128, CW], f32, tag="convps", name=f"cps{b}_{cb}_{ch}")
                    for ch in range(NCHUNKS)
                ]
                for t in range(TAPS):
                    dr, dc = t // 3, t % 3
                    lhs_hi = a8v[:, t, :, cb * 128 : cb * 128 + 128]
                    for ch in range(NCHUNKS):
                        off = (ch * RPC + dr) * PW + dc
                        for xi, rhs in enumerate(
                            (x8v[:, :, off : off + CW], xlov[:, :, off : off + CW])
                        ):
                            nc.tensor.matmul(
                                pss[ch][:, :],
                                lhsT=lhs_hi,
                                rhs=rhs,
                                start=(t == 0 and xi == 0),
                                stop=False,
                                perf_mode=DR,
                            )
                    if cb == 0 and t == 1 and hook is not None:
                        hook()
                for ch in range(NCHUNKS):
                    ps = pss[ch]
                    for t in range(TAPS):
                        dr, dc = t // 3, t % 3
                        off = (ch * RPC + dr) * PW + dc
                        nc.tensor.matmul(
                            ps[:, :],
                            lhsT=alov[:, t, :, cb * 128 : cb * 128 + 128],
                            rhs=x8v[:, :, off : off + CW],
                            start=False,
                            stop=(t == TAPS - 1),
                            perf_mode=DR,
                        )
                    ps3 = ps[:, :].rearrange("p (r c) -> p r c", r=RPC)
                    osb3 = osb[:, ch * RPC * W : (ch + 1) * RPC * W].rearrange(
                        "p (r c) -> p r c", r=RPC
                    )
                    nc.scalar.activation(
                        osb3[:, :, :], ps3[:, :, 0:W], AF.Copy, scale=1.0 / (SX * SW)
                    )
                ov = o_p[b, cb * 128 : (cb + 1) * 128].rearrange("co h w -> co (h w)")
                if b == B_PER_CORE - 1 and cb == 1:
                    # Tail: small trailing transfer so the kernel does not
                    # drain behind one long half-DMA after the last eviction.
                    cuts = (0, 1344, 2688, OUTF)
                    engs = (nc.sync, nc.gpsimd, nc.sync)
                    for s in range(3):
                        engs[s].dma_start(
                            out=ov[:, cuts[s] : cuts[s + 1]],
                            in_=osb[:, cuts[s] : cuts[s + 1]],
                        )
                else:
                    half = OUTF // 2
                    nc.sync.dma_start(out=ov[:, 0:half], in_=osb[:, 0:half])
                    nc.gpsimd.dma_start(out=ov[:, half:OUTF], in_=osb[:, half:OUTF])

        # Sample 0 prep runs in the open; later samples hide inside conv b-1.
        load_dma(0, nc.sync, nc.gpsimd, split_x8=True)
        warm_act()
        load_weights()
        stage_prep(0)
        load_dma(1, nc.sync, nc.gpsimd)
        stage_b(0, hook=lambda: stage_prep(1, xb=2))
        stage_b(1, hook=lambda: stage_prep(2, xb=3))
        stage_b(2, hook=lambda: stage_prep(3))
        stage_b(3)

    nc.compile()
    return nc


def _get_nc():
    if "nc" not in _cache:
        _cache["nc"] = _build_nc()
    return _cache["nc"]


def _make_in_maps(x, att_w, weight):
    import ml_dtypes

    E4 = ml_dtypes.float8_e4m3

    x = np.asarray(x, dtype=np.float32)
    att_w = np.asarray(att_w, dtype=np.float32)
    weight = np.asarray(weight, dtype=np.float32)

    # Host-side zero pad to (58, 58) + 4 tail elems, flattened per channel,
    # then exact hi+lo e4m3 split of x*SX at a shared scale.
    xp = np.zeros((B_TOTAL, CI, XT_F), dtype=np.float32)
    xp[:, :, :FLAT] = np.pad(
        x, ((0, 0), (0, 0), (1, 1), (1, 1))
    ).reshape(B_TOTAL, CI, FLAT)
    xs = xp * np.float32(SX)
    x8 = xs.astype(E4)
    xlo = (xs - x8.astype(np.float32)).astype(E4)

    # (B, ci, f) -> (B, p, two*f): partition p carries ci=p and ci=128+p.
    def to_two(a):
        return np.ascontiguousarray(
            a.reshape(B_TOTAL, TWO, 128, XT_F).transpose(0, 2, 1, 3)
        ).reshape(B_TOTAL, 128, TWO * XT_F)

    x8 = to_two(x8)
    xlo = to_two(xlo)

    # (K, Cout, Cin, kh, kw) -> [k, p, (tap, two, co)] bf16 for the mix.
    wt = weight.transpose(0, 2, 3, 4, 1)          # (k, ci, kh, kw, co)
    wt = wt.reshape(K, TWO, 128, TAPS, CO)        # (k, two, p, tap, co)
    wt = wt.transpose(0, 2, 3, 1, 4)              # (k, p, tap, two, co)
    wt = np.ascontiguousarray(wt).reshape(K, 128, MIXF).astype(ml_dtypes.bfloat16)

    # attrep[p, (c, k, j)] = att_w[k, c*128 + p] for all j.
    a = att_w.T.reshape(TWO, 128, K)              # (c, p, k)
    arep = np.repeat(a[:, :, :, None], 128, axis=3)   # (c, p, k, j)
    arep = np.ascontiguousarray(arep.transpose(1, 0, 2, 3)).reshape(
        128, TWO * K * 128
    ).astype(np.float32)

    return [
        {
            "x8": np.ascontiguousarray(x8[i * B_PER_CORE : (i + 1) * B_PER_CORE]),
            "xlo": np.ascontiguousarray(xlo[i * B_PER_CORE : (i + 1) * B_PER_CORE]),
            "w": wt,
            "attrep": arep,
        }
        for i in range(N_CORES)
    ]


def _run(x, att_w, weight, trace=False, **spmd_kwargs):
    from concourse.bass_utils import run_bass_kernel_spmd

    nc = _get_nc()
    in_maps = _make_in_maps(x, att_w, weight)
    res = run_bass_kernel_spmd(
        nc, in_maps, list(range(N_CORES)), trace=trace, **spmd_kwargs
    )
    out = np.concatenate([r["out"] for r in res.results], axis=0)
    return out.astype(np.float32, copy=False), res


def kernel(x, att_w, weight):
    out, _ = _run(x, att_w, weight)
    return out


# revision 13
# speedup vs baseline: 1.4002x; 1.0346x over previous
"""MoE-routed dynamic conv kernel for Trainium2 (8 NeuronCores, SPMD).

Problem: per-sample attention (global avg pool -> 1x1 conv -> sigmoid) mixes
K=4 expert 3x3 conv kernels; each sample is convolved with its own mixed
kernel.  x: (32, 256, 56, 56), att_w: (4, 256), weight: (4, 256, 256, 3, 3).

Strategy: data parallel over batch (4 samples per core, weights replicated).
The conv runs as an implicit GEMM in double-pumped fp8 (DoubleRow perf mode,
0.5 PE cycles per output row): each matmul instruction carries TWO
(weights, moving) fp8 pairs that are multiplied and summed in one pass, so
the two ci-blocks (ci 0-127 / 128-255) are contracted together.

Plain e4m3 quantization of both operands is too lossy (~3.5e-2 rel err vs
the 2e-2 gate), so both sides carry an exact hi+lo e4m3 split at a shared
power-of-2 scale (x*32, w*512) and each tap issues three DoubleRow matmuls:
    (w8 . x8), (w8 . xlo), (wlo . x8)        [wlo.xlo dropped, O(1e-3) rel]
i.e. 13.5 fp32r-equivalent passes instead of 18 -> ~0.75x PE time, with
measured end-to-end rel err ~1e-3.

x is zero-padded to (58, 58) on the host, quantized there into x8/xlo e4m3
planes laid out [p, two, flat] (two = ci-block), so every conv tap is a
contiguous [128, 2, 464] slice (8 padded rows); the two pad columns per row
are dropped by the strided PSUM->SBUF eviction, keeping the output DMA
contiguous.

Scheduling notes (the Tile scheduler list-schedules each engine greedily by
ready time; emission order only fixes ties, so the structure below shapes
READY TIMES, not queue positions):
  - The mix (agg = sum_k att_k w_k, bf16 mul/add tree on DVE; the fused
    scalar_tensor_tensor form runs 1x, tensor_scalar_mul 4x / tensor_tensor
    2x) and the agg8/agglo8 e4m3 hi/lo split are pipelined in TAP-THIRDS,
    and the hi conv passes run tap-outer, so sample 0's conv starts after
    only a third of the mix chain.
  - Pooling for sample 0 splits ci-block 0 to ACT (activation accum_out) /
    block 1 to a DVE reduce that finishes before the mix begins; for b>=1
    both halves run on ACT and are GATED behind agg8(b-1) by a tiny
    dependency op, or the greedy scheduler slots their 3.2us ops into the
    critical sigmoid->mix->agg8 chain of the previous sample.
  - No DMAs ride the ACT queue (its DGE path costs ~50% extra and blocks
    the in-order pool/sigmoid/evict stream).
  - A dummy sigmoid at t=0 preloads the activation-function table so the
    1.3us LoadActFuncSet is off the first sample's chain.
"""

import sys

if "/opt/trn_rl_repo" not in sys.path:
    sys.path.insert(0, "/opt/trn_rl_repo")

import numpy as np

B_TOTAL = 32
N_CORES = 8
B_PER_CORE = B_TOTAL // N_CORES  # 4
CI = 256
CO = 256
K = 4
H = W = 56
PH = PW = 58
FLAT = PH * PW            # 3364 padded image per ci-block
XT_F = FLAT + 4           # 3368: + tail pad for tap (2,2) overrun, host zeros
TWO = 2                   # ci-blocks paired per DoubleRow matmul
OUTF = H * W              # 3136 output cols per co-block (contiguous)
RPC = 8                   # output rows per PSUM chunk
CW = RPC * W              # 448 = 8 rows x 56 valid cols (dense PSUM chunk)
NCHUNKS = H // RPC        # 7
TAPS = 9
NT = 3                    # mix/quant pipeline stages (tap-thirds)
TPT = TAPS // NT          # 3 taps per third
THF = TPT * TWO * CO      # 1536 free elems per third
MIXF = TAPS * TWO * CO    # 4608 free elems per mixed-weight tile
SX = 32.0                 # x quant scale (power of 2; |x*32| < 240)
SW = 512.0                # w quant scale (power of 2; |agg*512| < 240)

_cache = {}


def _build_nc():
    from contextlib import ExitStack

    import concourse.bacc as bacc
    import concourse.mybir as mybir
    import concourse.tile as tile

    f32 = mybir.dt.float32
    bf16 = mybir.dt.bfloat16
    fp8 = mybir.dt.float8e4
    AF = mybir.ActivationFunctionType
    ALU = mybir.AluOpType
    DR = mybir.MatmulPerfMode.DoubleRow

    nc = bacc.Bacc("TRN2", target_bir_lowering=False, debug=False)
    x8_p = nc.declare_dram_parameter(
        "x8", [B_PER_CORE, 128, TWO * XT_F], fp8, isOutput=False
    )
    xlo_p = nc.declare_dram_parameter(
        "xlo", [B_PER_CORE, 128, TWO * XT_F], fp8, isOutput=False
    )
    w_p = nc.declare_dram_parameter("w", [K, 128, MIXF], bf16, isOutput=False)
    ar_p = nc.declare_dram_parameter("attrep", [128, TWO * K * 128], f32, isOutput=False)
    o_p = nc.declare_dram_parameter("out", [B_PER_CORE, CO, H, W], f32, isOutput=True)

    with ExitStack() as ctx:
        tc = ctx.enter_context(tile.TileContext(nc))
        pw = ctx.enter_context(tc.tile_pool(name="wpool", bufs=1))
        px = ctx.enter_context(tc.tile_pool(name="xpool", bufs=3))
        pagg = ctx.enter_context(tc.tile_pool(name="aggpool", bufs=2))
        pq = ctx.enter_context(tc.tile_pool(name="qpool", bufs=2))
        pout = ctx.enter_context(tc.tile_pool(name="outpool", bufs=2))
        psml = ctx.enter_context(tc.tile_pool(name="small", bufs=4))
        pps = ctx.enter_context(tc.tile_pool(name="cpsum", bufs=7, space="PSUM"))
        ppsa = ctx.enter_context(tc.tile_pool(name="apsum", bufs=1, space="PSUM"))

        ar_sb = pw.tile([128, TWO * K * 128], f32, tag="ar")
        w_sb = pw.tile([128, K * MIXF], bf16, tag="w", name="wt")
        # Mix scratch (bf16): reused across samples, WAR-serialized on DVE.
        m0 = pw.tile([128, MIXF], bf16, tag="m0")
        m1 = pw.tile([128, MIXF], bf16, tag="m1")
        psc = pw.tile([128, FLAT], fp8, tag="poolscr")
        warm = pw.tile([128, 2], f32, tag="warm")

        def warm_act():
            nc.vector.memset(warm[:, :], 0.0)
            nc.scalar.activation(warm[:, :], warm[:, :], AF.Sigmoid)

        def load_weights():
            nc.sync.dma_start(out=ar_sb[:, :], in_=ar_p[:, :])
            # Third j of expert k lands just before mix op (j, k) needs it.
            engs = (nc.sync, nc.gpsimd, nc.gpsimd, nc.sync)
            for j in range(NT):
                for k in range(K):
                    engs[k].dma_start(
                        out=w_sb[:, k * MIXF + j * THF : k * MIXF + (j + 1) * THF],
                        in_=w_p[k, :, j * THF : (j + 1) * THF],
                    )

        ar3 = ar_sb[:, :].rearrange("p (c rest) -> p c rest", c=TWO)

        state = {}
        agg8_of = {}

        def load_dma(b, x8_eng, xlo_eng, split_x8=False):
            x8t = px.tile([128, TWO * XT_F], fp8, tag="x8")
            xlot = px.tile([128, TWO * XT_F], fp8, tag="xlo")
            if split_x8:
                for c in range(TWO):
                    x8_eng.dma_start(
                        out=x8t[:, c * XT_F : (c + 1) * XT_F],
                        in_=x8_p[b, :, c * XT_F : (c + 1) * XT_F],
                    )
            else:
                x8_eng.dma_start(out=x8t[:, :], in_=x8_p[b, :, :])
            xlo_eng.dma_start(out=xlot[:, :], in_=xlo_p[b, :, :])
            state[b] = [x8t, xlot]

        def load_pool(b):
            """Pool x8 per ci-block.  b=0: ACT + DVE halves in parallel (the
            DVE finishes before the mix begins, so it cannot poison the mix
            chain).  b>=1: both halves on ACT, gated behind agg8(b-1)."""
            x8t, xlot = state[b]
            pl = psml.tile([128, TWO], f32, tag="pooled")
            x3 = x8t[:, :].rearrange("p (two f) -> p two f", two=TWO)
            if b > 0:
                nc.scalar.activation(
                    pl[:, :], agg8_of[b - 1][:, 3071:3073], AF.Copy, scale=0.0
                )
            nc.scalar.activation(
                psc[:, :], x3[:, 0, 0:FLAT], AF.Copy, accum_out=pl[:, 0:1]
            )
            if b == 0:
                nc.vector.tensor_reduce(
                    pl[:, 1:2],
                    x3[:, 1:2, 0:FLAT],
                    axis=mybir.AxisListType.X,
                    op=ALU.add,
                )
            else:
                nc.scalar.activation(
                    psc[:, :], x3[:, 1, 0:FLAT], AF.Copy, accum_out=pl[:, 1:2]
                )
            state[b] = [x8t, xlot, pl]

        def stage_att(b):
            """Attention matmuls + sigmoid for sample b (f32, tiny)."""
            x8t, xlot, pl = state[b]
            att_ps = ppsa.tile([128, K], f32, tag="attps")
            for k in range(K):
                for c in range(TWO):
                    nc.tensor.matmul(
                        att_ps[:, k : k + 1],
                        lhsT=ar3[:, c, k * 128 : (k + 1) * 128],
                        rhs=pl[:, c : c + 1],
                        start=(c == 0),
                        stop=(c == 1),
                    )
            att_sb = psml.tile([128, K], f32, tag="attsb")
            # pooled is computed from x*SX, so fold 1/SX into the logit scale.
            nc.scalar.activation(
                att_sb[:, :], att_ps[:, :], AF.Sigmoid, scale=1.0 / (H * W * SX)
            )
            state[b] = [x8t, xlot, att_sb]

        def stage_mix_quant(b):
            """Per tap-third: bf16 mul/add tree on DVE, then agg8 = e4m3(
            agg*SW) on ACT and agglo8 = agg*SW - agg8 on DVE."""
            x8t, xlot, att_sb = state[b]
            agg = pagg.tile([128, MIXF], bf16, tag="agg")
            agg8 = pq.tile([128, MIXF], fp8, tag="agg8")
            agglo = pq.tile([128, MIXF], fp8, tag="agglo")
            for j in range(NT):
                s = slice(j * THF, (j + 1) * THF)
                wv = [w_sb[:, k * MIXF + j * THF : k * MIXF + (j + 1) * THF]
                      for k in range(K)]
                nc.vector.tensor_scalar_mul(m0[:, s], wv[0], att_sb[:, 0:1])
                nc.vector.tensor_scalar_mul(m1[:, s], wv[1], att_sb[:, 1:2])
                nc.vector.tensor_tensor(agg[:, s], m0[:, s], m1[:, s], ALU.add)
                nc.vector.tensor_scalar_mul(m0[:, s], wv[2], att_sb[:, 2:3])
                nc.vector.tensor_scalar_mul(m1[:, s], wv[3], att_sb[:, 3:4])
                nc.vector.tensor_tensor(m0[:, s], m0[:, s], m1[:, s], ALU.add)
                nc.vector.tensor_tensor(agg[:, s], agg[:, s], m0[:, s], ALU.add)
                nc.scalar.activation(agg8[:, s], agg[:, s], AF.Copy, scale=SW)
                nc.vector.scalar_tensor_tensor(
                    agglo[:, s], agg[:, s], SW, agg8[:, s], ALU.mult, ALU.subtract
                )
            agg8_of[b] = agg8
            state[b] = [x8t, xlot, agg8, agglo]

        def stage_prep(b, xb=None):
            load_pool(b)
            stage_att(b)
            stage_mix_quant(b)
            if xb is not None:
                load_dma(xb, nc.sync, nc.gpsimd)

        def stage_b(b, hook=None):
            """Conv for sample b.

            Per co-block: hi passes (w8 . x8), (w8 . xlo) tap-outer (so the
            first tap-third of agg8 unblocks the PE), then the (wlo . x8)
            passes chunk-major + evictions."""
            x8t, xlot, agg8, agglo = state.pop(b)
            x8v = x8t[:, :].rearrange("p (two f) -> p two f", two=TWO)[
                :, :, 0:FLAT
            ].rearrange("p two (h w) -> p two h w", h=PH)
            xlov = xlot[:, :].rearrange("p (two f) -> p two f", two=TWO)[
                :, :, 0:FLAT
            ].rearrange("p two (h w) -> p two h w", h=PH)
            a8v = agg8[:, :].rearrange("p (t two co) -> p t two co", t=TAPS, two=TWO)
            alov = agglo[:, :].rearrange("p (t two co) -> p t two co", t=TAPS, two=TWO)
            for cb in range(2):
                osb = pout.tile([128, OUTF], f32, tag="osb")
                pss = [
                    pps.tile([128, CW], f32, tag="convps", name=f"cps{b}_{cb}_{ch}")
                    for ch in range(NCHUNKS)
                ]
                for t in range(TAPS):
                    dr, dc = t // 3, t % 3
                    lhs_hi = a8v[:, t, :, cb * 128 : cb * 128 + 128]
                    for ch in range(NCHUNKS):
                        r0 = ch * RPC + dr
                        for xi, rhs in enumerate(
                            (
                                x8v[:, :, r0 : r0 + RPC, dc : dc + W],
                                xlov[:, :, r0 : r0 + RPC, dc : dc + W],
                            )
                        ):
                            nc.tensor.matmul(
                                pss[ch][:, :],
                                lhsT=lhs_hi,
                                rhs=rhs,
                                start=(t == 0 and xi == 0),
                                stop=False,
                                perf_mode=DR,
                            )
                    if cb == 0 and t == 1 and hook is not None:
                        hook()
                for ch in range(NCHUNKS):
                    ps = pss[ch]
                    for t in range(TAPS):
                        dr, dc = t // 3, t % 3
                        r0 = ch * RPC + dr
                        nc.tensor.matmul(
                            ps[:, :],
                            lhsT=alov[:, t, :, cb * 128 : cb * 128 + 128],
                            rhs=x8v[:, :, r0 : r0 + RPC, dc : dc + W],
                            start=False,
                            stop=(t == TAPS - 1),
                            perf_mode=DR,
                        )
                    nc.scalar.activation(
                        osb[:, ch * CW : (ch + 1) * CW],
                        ps[:, :],
                        AF.Copy,
                        scale=1.0 / (SX * SW),
                    )
                ov = o_p[b, cb * 128 : (cb + 1) * 128].rearrange("co h w -> co (h w)")
                if b == B_PER_CORE - 1 and cb == 1:
                    # Tail: small trailing transfer so the kernel does not
                    # drain behind one long half-DMA after the last eviction.
                    cuts = (0, 1344, 2688, OUTF)
                    engs = (nc.sync, nc.gpsimd, nc.sync)
                    for s in range(3):
                        engs[s].dma_start(
                            out=ov[:, cuts[s] : cuts[s + 1]],
                            in_=osb[:, cuts[s] : cuts[s + 1]],
                        )
                else:
                    half = OUTF // 2
                    nc.sync.dma_start(out=ov[:, 0:half], in_=osb[:, 0:half])
                    nc.gpsimd.dma_start(out=ov[:, half:OUTF], in_=osb[:, half:OUTF])

        # Sample 0 prep runs in the open; later samples hide inside conv b-1.
        load_dma(0, nc.sync, nc.gpsimd, split_x8=True)
        warm_act()
        load_weights()
        stage_prep(0)
        load_dma(1, nc.sync, nc.gpsimd)
        stage_b(0, hook=lambda: stage_prep(1, xb=2))
        stage_b(1, hook=lambda: stage_prep(2, xb=3))
        stage_b(2, hook=lambda: stage_prep(3))
        stage_b(3)

    nc.compile()
    return nc


def _get_nc():
    if "nc" not in _cache:
        _cache["nc"] = _build_nc()
    return _cache["nc"]


def _make_in_maps(x, att_w, weight):
    import ml_dtypes

    E4 = ml_dtypes.float8_e4m3

    x = np.asarray(x, dtype=np.float32)
    att_w = np.asarray(att_w, dtype=np.float32)
    weight = np.asarray(weight, dtype=np.float32)

    # Host-side zero pad to (58, 58) + 4 tail elems, flattened per channel,
    # then exact hi+lo e4m3 split of x*SX at a shared scale.
    xp = np.zeros((B_TOTAL, CI, XT_F), dtype=np.float32)
    xp[:, :, :FLAT] = np.pad(
        x, ((0, 0), (0, 0), (1, 1), (1, 1))
    ).reshape(B_TOTAL, CI, FLAT)
    xs = xp * np.float32(SX)
    x8 = xs.astype(E4)
    xlo = (xs - x8.astype(np.float32)).astype(E4)

    # (B, ci, f) -> (B, p, two*f): partition p carries ci=p and ci=128+p.
    def to_two(a):
        return np.ascontiguousarray(
            a.reshape(B_TOTAL, TWO, 128, XT_F).transpose(0, 2, 1, 3)
        ).reshape(B_TOTAL, 128, TWO * XT_F)

    x8 = to_two(x8)
    xlo = to_two(xlo)

    # (K, Cout, Cin, kh, kw) -> [k, p, (tap, two, co)] bf16 for the mix.
    wt = weight.transpose(0, 2, 3, 4, 1)          # (k, ci, kh, kw, co)
    wt = wt.reshape(K, TWO, 128, TAPS, CO)        # (k, two, p, tap, co)
    wt = wt.transpose(0, 2, 3, 1, 4)              # (k, p, tap, two, co)
    wt = np.ascontiguousarray(wt).reshape(K, 128, MIXF).astype(ml_dtypes.bfloat16)

    # attrep[p, (c, k, j)] = att_w[k, c*128 + p] for all j.
    a = att_w.T.reshape(TWO, 128, K)              # (c, p, k)
    arep = np.repeat(a[:, :, :, None], 128, axis=3)   # (c, p, k, j)
    arep = np.ascontiguousarray(arep.transpose(1, 0, 2, 3)).reshape(
        128, TWO * K * 128
    ).astype(np.float32)

    return [
        {
            "x8": np.ascontiguousarray(x8[i * B_PER_CORE : (i + 1) * B_PER_CORE]),
            "xlo": np.ascontiguousarray(xlo[i * B_PER_CORE : (i + 1) * B_PER_CORE]),
            "w": wt,
            "attrep": arep,
        }
        for i in range(N_CORES)
    ]


def _run(x, att_w, weight, trace=False, **spmd_kwargs):
    from concourse.bass_utils import run_bass_kernel_spmd

    nc = _get_nc()
    in_maps = _make_in_maps(x, att_w, weight)
    res = run_bass_kernel_spmd(
        nc, in_maps, list(range(N_CORES)), trace=trace, **spmd_kwargs
    )
    out = np.concatenate([r["out"] for r in res.results], axis=0)
    return out.astype(np.float32, copy=False), res


def kernel(x, att_w, weight):
    out, _ = _run(x, att_w, weight)
    return out


# revision 16
# speedup vs baseline: 1.4049x; 1.0033x over previous
"""MoE-routed dynamic conv kernel for Trainium2 (8 NeuronCores, SPMD).

Problem: per-sample attention (global avg pool -> 1x1 conv -> sigmoid) mixes
K=4 expert 3x3 conv kernels; each sample is convolved with its own mixed
kernel.  x: (32, 256, 56, 56), att_w: (4, 256), weight: (4, 256, 256, 3, 3).

Strategy: data parallel over batch (4 samples per core, weights replicated).
The conv runs as an implicit GEMM in double-pumped fp8 (DoubleRow perf mode,
0.5 PE cycles per output row): each matmul instruction carries TWO
(weights, moving) fp8 pairs that are multiplied and summed in one pass, so
the two ci-blocks (ci 0-127 / 128-255) are contracted together.

Plain e4m3 quantization of both operands is too lossy (~3.5e-2 rel err vs
the 2e-2 gate), so both sides carry an exact hi+lo e4m3 split at a shared
power-of-2 scale (x*32, w*512) and each tap issues three DoubleRow matmuls:
    (w8 . x8), (w8 . xlo), (wlo . x8)        [wlo.xlo dropped, O(1e-3) rel]
i.e. 13.5 fp32r-equivalent passes instead of 18 -> ~0.75x PE time, with
measured end-to-end rel err ~1e-3.

x is zero-padded to (58, 58) on the host, quantized there into x8/xlo e4m3
planes laid out [p, two, flat] (two = ci-block), so every conv tap is a
contiguous [128, 2, 464] slice (8 padded rows); the two pad columns per row
are dropped by the strided PSUM->SBUF eviction, keeping the output DMA
contiguous.

Scheduling notes (the Tile scheduler list-schedules each engine greedily by
ready time; emission order only fixes ties, so the structure below shapes
READY TIMES, not queue positions):
  - The mix (agg = sum_k att_k w_k, bf16 mul/add tree on DVE; the fused
    scalar_tensor_tensor form runs 1x, tensor_scalar_mul 4x / tensor_tensor
    2x) and the agg8/agglo8 e4m3 hi/lo split are pipelined in TAP-THIRDS,
    and the hi conv passes run tap-outer, so sample 0's conv starts after
    only a third of the mix chain.
  - Pooling for sample 0 splits ci-block 0 to ACT (activation accum_out) /
    block 1 to a DVE reduce that finishes before the mix begins; for b>=1
    both halves run on ACT and are GATED behind agg8(b-1) by a tiny
    dependency op, or the greedy scheduler slots their 3.2us ops into the
    critical sigmoid->mix->agg8 chain of the previous sample.
  - No DMAs ride the ACT queue (its DGE path costs ~50% extra and blocks
    the in-order pool/sigmoid/evict stream).
  - A dummy sigmoid at t=0 preloads the activation-function table so the
    1.3us LoadActFuncSet is off the first sample's chain.
"""

import sys

if "/opt/trn_rl_repo" not in sys.path:
    sys.path.insert(0, "/opt/trn_rl_repo")

import numpy as np

B_TOTAL = 32
N_CORES = 8
B_PER_CORE = B_TOTAL // N_CORES  # 4
CI = 256
CO = 256
K = 4
H = W = 56
PH = PW = 58
FLAT = PH * PW            # 3364 padded image per ci-block
XT_F = FLAT + 4           # 3368: + tail pad for tap (2,2) overrun, host zeros
TWO = 2                   # ci-blocks paired per DoubleRow matmul
OUTF = H * W              # 3136 output cols per co-block (contiguous)
RPC = 8                   # output rows per PSUM chunk
CW = RPC * W              # 448 = 8 rows x 56 valid cols (dense PSUM chunk)
NCHUNKS = H // RPC        # 7
TAPS = 9
NT = 3                    # mix/quant pipeline stages (tap-thirds)
TPT = TAPS // NT          # 3 taps per third
THF = TPT * TWO * CO      # 1536 free elems per third
MIXF = TAPS * TWO * CO    # 4608 free elems per mixed-weight tile
SX = 32.0                 # x quant scale (power of 2; |x*32| < 240)
SW = 512.0                # w quant scale (power of 2; |agg*512| < 240)

_cache = {}


def _build_nc():
    from contextlib import ExitStack

    import concourse.bacc as bacc
    import concourse.mybir as mybir
    import concourse.tile as tile

    f32 = mybir.dt.float32
    bf16 = mybir.dt.bfloat16
    fp8 = mybir.dt.float8e4
    AF = mybir.ActivationFunctionType
    ALU = mybir.AluOpType
    DR = mybir.MatmulPerfMode.DoubleRow

    nc = bacc.Bacc("TRN2", target_bir_lowering=False, debug=False)
    x8_p = nc.declare_dram_parameter(
        "x8", [B_PER_CORE, 128, TWO * XT_F], fp8, isOutput=False
    )
    xlo_p = nc.declare_dram_parameter(
        "xlo", [B_PER_CORE, 128, TWO * XT_F], fp8, isOutput=False
    )
    w_p = nc.declare_dram_parameter("w", [K, 128, MIXF], bf16, isOutput=False)
    ar_p = nc.declare_dram_parameter("attrep", [128, TWO * K * 128], f32, isOutput=False)
    o_p = nc.declare_dram_parameter("out", [B_PER_CORE, CO, H, W], f32, isOutput=True)

    with ExitStack() as ctx:
        tc = ctx.enter_context(tile.TileContext(nc))
        pw = ctx.enter_context(tc.tile_pool(name="wpool", bufs=1))
        px = ctx.enter_context(tc.tile_pool(name="xpool", bufs=3))
        pagg = ctx.enter_context(tc.tile_pool(name="aggpool", bufs=2))
        pq = ctx.enter_context(tc.tile_pool(name="qpool", bufs=2))
        pout = ctx.enter_context(tc.tile_pool(name="outpool", bufs=2))
        psml = ctx.enter_context(tc.tile_pool(name="small", bufs=4))
        pps = ctx.enter_context(tc.tile_pool(name="cpsum", bufs=7, space="PSUM"))
        ppsa = ctx.enter_context(tc.tile_pool(name="apsum", bufs=1, space="PSUM"))

        ar_sb = pw.tile([128, TWO * K * 128], f32, tag="ar")
        w_sb = pw.tile([128, K * MIXF], bf16, tag="w", name="wt")
        # Mix scratch (bf16): reused across samples, WAR-serialized on DVE.
        m0 = pw.tile([128, MIXF], bf16, tag="m0")
        m1 = pw.tile([128, MIXF], bf16, tag="m1")
        psc = pw.tile([128, FLAT], fp8, tag="poolscr")
        warm = pw.tile([128, 2], f32, tag="warm")

        def warm_act():
            nc.vector.memset(warm[:, :], 0.0)
            nc.scalar.activation(warm[:, :], warm[:, :], AF.Sigmoid)

        def load_weights():
            nc.sync.dma_start(out=ar_sb[:, :], in_=ar_p[:, :])
            # Third j of expert k lands just before mix op (j, k) needs it.
            engs = (nc.sync, nc.gpsimd, nc.gpsimd, nc.sync)
            for j in range(NT):
                for k in range(K):
                    engs[k].dma_start(
                        out=w_sb[:, k * MIXF + j * THF : k * MIXF + (j + 1) * THF],
                        in_=w_p[k, :, j * THF : (j + 1) * THF],
                    )

        ar3 = ar_sb[:, :].rearrange("p (c rest) -> p c rest", c=TWO)

        state = {}
        agg8_of = {}

        def load_dma(b, x8_eng, xlo_eng, split_x8=False):
            x8t = px.tile([128, TWO * XT_F], fp8, tag="x8")
            xlot = px.tile([128, TWO * XT_F], fp8, tag="xlo")
            if split_x8:
                for c in range(TWO):
                    x8_eng.dma_start(
                        out=x8t[:, c * XT_F : (c + 1) * XT_F],
                        in_=x8_p[b, :, c * XT_F : (c + 1) * XT_F],
                    )
            else:
                x8_eng.dma_start(out=x8t[:, :], in_=x8_p[b, :, :])
            xlo_eng.dma_start(out=xlot[:, :], in_=xlo_p[b, :, :])
            state[b] = [x8t, xlot]

        def load_pool(b):
            """Pool x8 per ci-block.  b=0: ACT + DVE halves in parallel (the
            DVE finishes before the mix begins, so it cannot poison the mix
            chain).  b>=1: both halves on ACT, gated behind agg8(b-1)."""
            x8t, xlot = state[b]
            pl = psml.tile([128, TWO], f32, tag="pooled")
            x3 = x8t[:, :].rearrange("p (two f) -> p two f", two=TWO)
            if b > 0:
                nc.scalar.activation(
                    pl[:, :], agg8_of[b - 1][:, 3071:3073], AF.Copy, scale=0.0
                )
            nc.scalar.activation(
                psc[:, :], x3[:, 0, 0:FLAT], AF.Copy, accum_out=pl[:, 0:1]
            )
            if b == 0:
                nc.vector.tensor_reduce(
                    pl[:, 1:2],
                    x3[:, 1:2, 0:FLAT],
                    axis=mybir.AxisListType.X,
                    op=ALU.add,
                )
            else:
                nc.scalar.activation(
                    psc[:, :], x3[:, 1, 0:FLAT], AF.Copy, accum_out=pl[:, 1:2]
                )
            state[b] = [x8t, xlot, pl]

        def stage_att(b):
            """Attention matmuls + sigmoid for sample b (f32, tiny)."""
            x8t, xlot, pl = state[b]
            att_ps = ppsa.tile([128, K], f32, tag="attps")
            for k in range(K):
                for c in range(TWO):
                    nc.tensor.matmul(
                        att_ps[:, k : k + 1],
                        lhsT=ar3[:, c, k * 128 : (k + 1) * 128],
                        rhs=pl[:, c : c + 1],
                        start=(c == 0),
                        stop=(c == 1),
                    )
            att_sb = psml.tile([128, K], f32, tag="attsb")
            # pooled is computed from x*SX, so fold 1/SX into the logit scale.
            nc.scalar.activation(
                att_sb[:, :], att_ps[:, :], AF.Sigmoid, scale=1.0 / (H * W * SX)
            )
            state[b] = [x8t, xlot, att_sb]

        def stage_mix_quant(b):
            """Per tap-third: bf16 mul/add tree on DVE, then agg8 = e4m3(
            agg*SW) on ACT and agglo8 = agg*SW - agg8 on DVE."""
            x8t, xlot, att_sb = state[b]
            agg = pagg.tile([128, MIXF], bf16, tag="agg")
            agg8 = pq.tile([128, MIXF], fp8, tag="agg8")
            agglo = pq.tile([128, MIXF], fp8, tag="agglo")
            for j in range(NT):
                s = slice(j * THF, (j + 1) * THF)
                wv = [w_sb[:, k * MIXF + j * THF : k * MIXF + (j + 1) * THF]
                      for k in range(K)]
                nc.vector.tensor_scalar_mul(m0[:, s], wv[0], att_sb[:, 0:1])
                nc.vector.tensor_scalar_mul(m1[:, s], wv[1], att_sb[:, 1:2])
                nc.vector.tensor_tensor(agg[:, s], m0[:, s], m1[:, s], ALU.add)
                nc.vector.tensor_scalar_mul(m0[:, s], wv[2], att_sb[:, 2:3])
                nc.vector.tensor_scalar_mul(m1[:, s], wv[3], att_sb[:, 3:4])
                nc.vector.tensor_tensor(m0[:, s], m0[:, s], m1[:, s], ALU.add)
                nc.vector.tensor_tensor(agg[:, s], agg[:, s], m0[:, s], ALU.add)
                nc.scalar.activation(agg8[:, s], agg[:, s], AF.Copy, scale=SW)
                nc.vector.scalar_tensor_tensor(
                    agglo[:, s], agg[:, s], SW, agg8[:, s], ALU.mult, ALU.subtract
                )
            agg8_of[b] = agg8
            state[b] = [x8t, xlot, agg8, agglo]

        def stage_prep(b, xb=None):
            load_pool(b)
            stage_att(b)
            stage_mix_quant(b)
            if xb is not None:
                load_dma(xb, nc.sync, nc.gpsimd)

        def stage_b(b, hook=None):
            """Conv for sample b.

            Per co-block: hi passes (w8 . x8), (w8 . xlo) tap-outer (so the
            first tap-third of agg8 unblocks the PE), then the (wlo . x8)
            passes chunk-major + evictions."""
            x8t, xlot, agg8, agglo = state.pop(b)
            x8v = x8t[:, :].rearrange("p (two f) -> p two f", two=TWO)[
                :, :, 0:FLAT
            ].rearrange("p two (h w) -> p two h w", h=PH)
            xlov = xlot[:, :].rearrange("p (two f) -> p two f", two=TWO)[
                :, :, 0:FLAT
            ].rearrange("p two (h w) -> p two h w", h=PH)
            a8v = agg8[:, :].rearrange("p (t two co) -> p t two co", t=TAPS, two=TWO)
            alov = agglo[:, :].rearrange("p (t two co) -> p t two co", t=TAPS, two=TWO)
            for cb in range(2):
                osb = pout.tile([128, OUTF], f32, tag="osb")
                pss = [
                    pps.tile([128, CW], f32, tag="convps", name=f"cps{b}_{cb}_{ch}")
                    for ch in range(NCHUNKS)
                ]
                for t in range(TAPS):
                    dr, dc = t // 3, t % 3
                    lhs_hi = a8v[:, t, :, cb * 128 : cb * 128 + 128]
                    for ch in range(NCHUNKS):
                        r0 = ch * RPC + dr
                        for xi, rhs in enumerate(
                            (
                                x8v[:, :, r0 : r0 + RPC, dc : dc + W],
                                xlov[:, :, r0 : r0 + RPC, dc : dc + W],
                            )
                        ):
                            nc.tensor.matmul(
                                pss[ch][:, :],
                                lhsT=lhs_hi,
                                rhs=rhs,
                                start=(t == 0 and xi == 0),
                                stop=False,
                                perf_mode=DR,
                            )
                    if cb == 0 and t == 1 and hook is not None:
                        hook()
                for ch in range(NCHUNKS):
                    ps = pss[ch]
                    for t in range(TAPS):
                        dr, dc = t // 3, t % 3
                        r0 = ch * RPC + dr
                        nc.tensor.matmul(
                            ps[:, :],
                            lhsT=alov[:, t, :, cb * 128 : cb * 128 + 128],
                            rhs=x8v[:, :, r0 : r0 + RPC, dc : dc + W],
                            start=False,
                            stop=(t == TAPS - 1),
                            perf_mode=DR,
                        )
                    nc.scalar.activation(
                        osb[:, ch * CW : (ch + 1) * CW],
                        ps[:, :],
                        AF.Copy,
                        scale=1.0 / (SX * SW),
                    )
                ov = o_p[b, cb * 128 : (cb + 1) * 128].rearrange("co h w -> co (h w)")
                if b == B_PER_CORE - 1 and cb == 1:
                    # Tail: small trailing transfer so the kernel does not
                    # drain behind one long half-DMA after the last eviction.
                    cuts = (0, 1344, 2016, 2688, OUTF)
                    engs = (nc.sync, nc.gpsimd, nc.sync, nc.gpsimd)
                    for s in range(4):
                        engs[s].dma_start(
                            out=ov[:, cuts[s] : cuts[s + 1]],
                            in_=osb[:, cuts[s] : cuts[s + 1]],
                        )
                else:
                    half = OUTF // 2
                    nc.sync.dma_start(out=ov[:, 0:half], in_=osb[:, 0:half])
                    nc.gpsimd.dma_start(out=ov[:, half:OUTF], in_=osb[:, half:OUTF])

        # Sample 0 prep runs in the open; later samples hide inside conv b-1.
        load_dma(0, nc.sync, nc.gpsimd, split_x8=True)
        warm_act()
        load_weights()
        stage_prep(0)
        load_dma(1, nc.sync, nc.gpsimd)
        stage_b(0, hook=lambda: stage_prep(1, xb=2))
        stage_b(1, hook=lambda: stage_prep(2, xb=3))
        stage_b(2, hook=lambda: stage_prep(3))
        stage_b(3)

    nc.compile()
    return nc


def _get_nc():
    if "nc" not in _cache:
        _cache["nc"] = _build_nc()
    return _cache["nc"]


def _make_in_maps(x, att_w, weight):
    import ml_dtypes

    E4 = ml_dtypes.float8_e4m3

    x = np.asarray(x, dtype=np.float32)
    att_w = np.asarray(att_w, dtype=np.float32)
    weight = np.asarray(weight, dtype=np.float32)

    # Host-side zero pad to (58, 58) + 4 tail elems, flattened per channel,
    # then exact hi+lo e4m3 split of x*SX at a shared scale.
    xp = np.zeros((B_TOTAL, CI, XT_F), dtype=np.float32)
    xp[:, :, :FLAT] = np.pad(
        x, ((0, 0), (0, 0), (1, 1), (1, 1))
    ).reshape(B_TOTAL, CI, FLAT)
    xs = xp * np.float32(SX)
    x8 = xs.astype(E4)
    xlo = (xs - x8.astype(np.float32)).astype(E4)

    # (B, ci, f) -> (B, p, two*f): partition p carries ci=p and ci=128+p.
    def to_two(a):
        return np.ascontiguousarray(
            a.reshape(B_TOTAL, TWO, 128, XT_F).transpose(0, 2, 1, 3)
        ).reshape(B_TOTAL, 128, TWO * XT_F)

    x8 = to_two(x8)
    xlo = to_two(xlo)

    # (K, Cout, Cin, kh, kw) -> [k, p, (tap, two, co)] bf16 for the mix.
    wt = weight.transpose(0, 2, 3, 4, 1)          # (k, ci, kh, kw, co)
    wt = wt.reshape(K, TWO, 128, TAPS, CO)        # (k, two, p, tap, co)
    wt = wt.transpose(0, 2, 3, 1, 4)              # (k, p, tap, two, co)
    wt = np.ascontiguousarray(wt).reshape(K, 128, MIXF).astype(ml_dtypes.bfloat16)

    # attrep[p, (c, k, j)] = att_w[k, c*128 + p] for all j.
    a = att_w.T.reshape(TWO, 128, K)              # (c, p, k)
    arep = np.repeat(a[:, :, :, None], 128, axis=3)   # (c, p, k, j)
    arep = np.ascontiguousarray(arep.transpose(1, 0, 2, 3)).reshape(
        128, TWO * K * 128
    ).astype(np.float32)

    return [
        {
            "x8": np.ascontiguousarray(x8[i * B_PER_CORE : (i + 1) * B_PER_CORE]),
            "xlo": np.ascontiguousarray(xlo[i * B_PER_CORE : (i + 1) * B_PER_CORE]),
            "w": wt,
            "attrep": arep,
        }
        for i in range(N_CORES)
    ]


def _run(x, att_w, weight, trace=False, **spmd_kwargs):
    from concourse.bass_utils import run_bass_kernel_spmd

    nc = _get_nc()
    in_maps = _make_in_maps(x, att_w, weight)
    res = run_bass_kernel_spmd(
        nc, in_maps, list(range(N_CORES)), trace=trace, **spmd_kwargs
    )
    out = np.concatenate([r["out"] for r in res.results], axis=0)
    return out.astype(np.float32, copy=False), res


def kernel(x, att_w, weight):
    out, _ = _run(x, att_w, weight)
    return out
